# revision 1
# baseline (speedup 1.0000x reference)
"""MoE router gate kernel for Trainium2 (Bass/Tile), 8-core data-parallel.

Computes, for x[16384, 7168], weight[256, 7168], bias[256]:
    scores  = sigmoid(x @ weight.T)
    biased  = scores + bias
    indices = top8(biased)                        (descending, int32)
    weights = scores[indices] / sum * 2.5         (float32)

Sharding: data-parallel over tokens (2048 tokens/core), weight/bias
replicated.  Host pre-arranges x into a transposed tiled layout so the
contraction dim lands on SBUF partitions with contiguous DMAs.

Default variant "k3": x is shipped as xh=fp16(x*16) (2B) plus the fp8
residual xl8 (1B) — 3 bytes/element of DMA.  The fp16 main matmul
accumulates xh*wh; a DoubleRow fp8 matmul accumulates the two
correction terms fp8(xh)*wl8 + xl8*wh8, where fp8(xh) is derived
on-device by an Activation-engine cast so it costs no HBM traffic.
Score error ~2^-15: top-8 selection matches the fp32 reference on all
but ~10 of 16384 rows (rel err 4.7e-3 vs the 2e-2 gate; plain fp16 or
f32r fails the gate at 2.3-3.7e-2).

Measured on the axon TRN2 cores the kernel is PE-bound at the moving-
operand stream rate (~1 fp16 row or 2 packed fp8 rows per cycle at
~2GHz): main pass + correction pass = 458752 PE cycles = ~229us/rep,
which the measured slope matches; DMA (3B/elem = ~123us), the ACT cast
(~105us) and the DVE epilogue (~78us) all hide underneath.  Under the
CoreSim cost model the same program is ~149us/rep (DMA-bound).
"""

import os
from concurrent.futures import ThreadPoolExecutor

import numpy as np

TOKENS = 16384
DIM = 7168
NEXP = 256
TOPK = 8
ROUTE_SCALE = 2.5
NCORES = 8
TPC = TOKENS // NCORES          # tokens per core: 2048
P = 128                         # partitions / tile height
NTILES = TPC // P               # 16 token tiles per core
KC = DIM // P                   # 56 contraction chunks

# Matmul variant:
#   "k3"      fp16 main + fp8 DoubleRow corrections, xh8 cast on-device (3B/elem DMA)
#   "hyb4"    same math, xh8 shipped from host (4B/elem DMA)
#   "fp32"    exact, 4 cyc/row
#   "f32r"    1 cyc/row, tf32-like (fails the index gate; kept for probing)
MM_DTYPE = os.environ.get("GATE_MM_DTYPE", "k3")
# Perf probes: "dr2x" duplicates the DoubleRow pass, "main2x" duplicates the
# fp16 pass (results merged at 1e-30 weight so values stay valid).  The slope
# delta vs plain k3 isolates each pass's true hardware cost.
PROBE = os.environ.get("GATE_PROBE", "")
# "alt" (default) interleaves fp16/DoubleRow matmuls so stationary loads
# hide in the PE's spare weight bank; "split" runs them as two passes.
MM_ORDER = os.environ.get("GATE_MM_ORDER", "alt")
XBUFS = int(os.environ.get("GATE_XBUFS", "3"))
LOOKAHEAD = int(os.environ.get("GATE_LOOKAHEAD", "2"))
# 4 PSUM buffers per accumulation tag (8 half-banks of 8): lets the PE run
# a tile further ahead of the DVE evacuation chain — measured ~215us vs
# ~230us with 3 (the evacuation sits behind sigmoid in the ACT/DVE queues
# and occasionally delays the bank release enough to stall the PE).
PSBUFS = int(os.environ.get("GATE_PSBUFS", "4"))
X_SCALE = 16.0   # keep x_lo out of fp16-denormal range
W_SCALE = 64.0   # keep w_lo out of fp16-denormal range
S_XL = 512.0     # scale of fp8(x residual)
S_WH = 8.0       # scale of fp8(w)
S_WL = S_XL * S_WH * 1.0  # scale of fp8(w residual); must equal S_XL*S_WH


def _build_program(reps=1):
    import concourse.bacc as bacc
    import concourse.mybir as mybir
    import concourse.tile as tile

    f32 = mybir.dt.float32
    f16 = mybir.dt.float16
    f8 = mybir.dt.float8e4
    u32 = mybir.dt.uint32
    hyb = MM_DTYPE in ("k3", "hyb4")
    cast_dev = MM_DTYPE == "k3"
    mm_dt = None if hyb else {
        "fp32": mybir.dt.float32,
        "f32r": mybir.dt.float32r,
    }[MM_DTYPE]
    sig_scale = 1.0 / (X_SCALE * W_SCALE) if hyb else 1.0

    nc = bacc.Bacc(
        "TRN2",
        target_bir_lowering=False,
        debug=False,
        enable_asserts=False,
        num_devices=NCORES,
    )

    if hyb:
        xh_d = nc.dram_tensor("xh", [NTILES, P, KC, P], f16, kind="ExternalInput").ap()
        # residual fp8: one contiguous [P, KC*P] block per tile
        xl8_d = nc.dram_tensor(
            "xl8", [NTILES, P, KC, P], f8, kind="ExternalInput"
        ).ap()
        if not cast_dev:
            xh8_d = nc.dram_tensor(
                "xh8", [NTILES, P, KC, P], f8, kind="ExternalInput"
            ).ap()
        wh_d = nc.dram_tensor("wh", [P, KC, NEXP], f16, kind="ExternalInput").ap()
        # w8[:, 0] = wl8 (pairs xh8), w8[:, 1] = wh8 (pairs xl8)
        w8_d = nc.dram_tensor("w8", [P, 2, KC, NEXP], f8, kind="ExternalInput").ap()
    else:
        xt_d = nc.dram_tensor(
            "xt", [NTILES, P, KC, P], mm_dt, kind="ExternalInput"
        ).ap()
        wt_d = nc.dram_tensor("wt", [P, KC, NEXP], mm_dt, kind="ExternalInput").ap()
    bb_d = nc.dram_tensor("bb", [P, NEXP], f32, kind="ExternalInput").ap()
    ow_d = nc.dram_tensor("out_w", [NTILES, P, TOPK], f32, kind="ExternalOutput").ap()
    oi_d = nc.dram_tensor("out_i", [NTILES, P, TOPK], u32, kind="ExternalOutput").ap()

    with tile.TileContext(nc) as tc:
        with (
            tc.tile_pool(name="const", bufs=1) as const_pool,
            tc.tile_pool(name="xin", bufs=XBUFS) as x_pool,
            tc.tile_pool(name="psum", bufs=PSBUFS, space="PSUM") as ps_pool,
            tc.tile_pool(name="epi", bufs=3) as ep_pool,
        ):
            if hyb:
                wh_sb = const_pool.tile([P, KC, NEXP], f16)
                nc.sync.dma_start(wh_sb[:], wh_d)
                w8_sb = const_pool.tile([P, 2, KC, NEXP], f8)
                nc.sync.dma_start(w8_sb[:], w8_d)
            else:
                wt_sb = const_pool.tile([P, KC, NEXP], mm_dt)
                nc.sync.dma_start(wt_sb[:], wt_d)
            bb_sb = const_pool.tile([P, NEXP], f32)
            nc.sync.dma_start(bb_sb[:], bb_d)

            if PROBE in ("pe", "pefp16"):
                # PE-isolation probe: one resident x tile, matmul mix only,
                # minimal PSUM evacuation, everything else stripped.
                xh_sb = x_pool.tile([P, KC, P], f16, tag="xh")
                nc.sync.dma_start(xh_sb[:], xh_d[0])
                x8_sb = x_pool.tile([P, 2, KC, P], f8, tag="x8")
                nc.sync.dma_start(x8_sb[:, 1], xl8_d[0])
                nc.scalar.activation(
                    x8_sb[:, 0], xh_sb[:], mybir.ActivationFunctionType.Copy
                )
                dbg_d = nc.dram_tensor(
                    "dbg", [NTILES, P, NEXP], f32, kind="ExternalOutput"
                ).ap()
                for b in [b for _ in range(reps) for b in range(NTILES)]:
                    ps = ps_pool.tile([P, NEXP], f32, tag="ps")
                    psc = None
                    if PROBE == "pe":
                        psc = ps_pool.tile([P, NEXP], f32, tag="psc")
                    for k in range(KC):
                        if MM_ORDER == "split" and psc is not None:
                            continue
                        nc.tensor.matmul(
                            ps[:], xh_sb[:, k, :], wh_sb[:, k, :],
                            start=(k == 0), stop=(k == KC - 1),
                        )
                        if psc is not None and MM_ORDER != "split":
                            nc.tensor.matmul(
                                psc[:], x8_sb[:, :, k, :], w8_sb[:, :, k, :],
                                start=(k == 0), stop=(k == KC - 1),
                                perf_mode=mybir.MatmulPerfMode.DoubleRow,
                            )
                    if MM_ORDER == "split" and psc is not None:
                        for k in range(KC):
                            nc.tensor.matmul(
                                ps[:], xh_sb[:, k, :], wh_sb[:, k, :],
                                start=(k == 0), stop=(k == KC - 1),
                            )
                        for k in range(KC):
                            nc.tensor.matmul(
                                psc[:], x8_sb[:, :, k, :], w8_sb[:, :, k, :],
                                start=(k == 0), stop=(k == KC - 1),
                                perf_mode=mybir.MatmulPerfMode.DoubleRow,
                            )
                    comb = ep_pool.tile([P, NEXP], f32, tag="comb")
                    if psc is not None:
                        corr = ep_pool.tile([P, NEXP], f32, tag="corr")
                        nc.vector.tensor_scalar(
                            corr[:], psc[:], 1.0 / S_WL, None,
                            op0=mybir.AluOpType.mult,
                        )
                        nc.vector.tensor_add(comb[:], ps[:], corr[:])
                    else:
                        nc.vector.tensor_scalar(
                            comb[:], ps[:], 1.0, None, op0=mybir.AluOpType.mult
                        )
                    nc.sync.dma_start(dbg_d[b], comb[:])
                nc.compile()
                return nc

            seq = [b for _ in range(reps) for b in range(NTILES)]
            loaded = []

            def issue_load(b):
                """DMA (and cast) the inputs for token tile b.  Called one
                iteration ahead so the ACT-engine cast for tile b+1 sits in
                front of tile b's sigmoid in the ACT queue — otherwise the
                next tile's DoubleRow matmuls stall on a cast that is stuck
                behind an epilogue dependent on this tile's matmuls."""
                if hyb:
                    xh_sb = x_pool.tile([P, KC, P], f16, tag="xh")
                    nc.sync.dma_start(xh_sb[:], xh_d[b])
                    # x8[:, 0] = fp8(xh) (pairs wl8), x8[:, 1] = xl8 (pairs wh8)
                    x8_sb = x_pool.tile([P, 2, KC, P], f8, tag="x8")
                    nc.sync.dma_start(x8_sb[:, 1], xl8_d[b])
                    if cast_dev:
                        nc.scalar.activation(
                            x8_sb[:, 0],
                            xh_sb[:],
                            mybir.ActivationFunctionType.Copy,
                        )
                    else:
                        nc.sync.dma_start(x8_sb[:, 0], xh8_d[b])
                    loaded.append((xh_sb, x8_sb))
                else:
                    xt_sb = x_pool.tile([P, KC, P], mm_dt, tag="xt")
                    nc.sync.dma_start(xt_sb[:], xt_d[b])
                    loaded.append((xt_sb,))

            for j in range(min(LOOKAHEAD, len(seq))):
                issue_load(seq[j])
            for i, b in enumerate(seq):
                if i + LOOKAHEAD < len(seq):
                    issue_load(seq[i + LOOKAHEAD])
                ps = ps_pool.tile([P, NEXP], f32, tag="ps")
                if hyb:
                    xh_sb, x8_sb = loaded.pop(0)
                    psc = ps_pool.tile([P, NEXP], f32, tag="psc")
                    # Alternate the fp16 and DoubleRow chains: the fp16
                    # matmul's 256-cycle moving pass covers the DR pair's
                    # 256-row stationary load in the PE's second weight
                    # bank, and vice versa (128 vs 128).  Splitting into two
                    # separate passes stalls the DR chain on its own loads.
                    if MM_ORDER == "split":
                        for k in range(KC):
                            nc.tensor.matmul(
                                ps[:],
                                xh_sb[:, k, :],
                                wh_sb[:, k, :],
                                start=(k == 0),
                                stop=(k == KC - 1),
                            )
                        for k in range(KC):
                            nc.tensor.matmul(
                                psc[:],
                                x8_sb[:, :, k, :],
                                w8_sb[:, :, k, :],
                                start=(k == 0),
                                stop=(k == KC - 1),
                                perf_mode=mybir.MatmulPerfMode.DoubleRow,
                            )
                    elif MM_ORDER == "alt2":
                        # 2:1 interleave — each DR stationary load gets two
                        # fp16 moving passes (512 cycles) of cover
                        for k in range(0, KC, 2):
                            nc.tensor.matmul(
                                ps[:], xh_sb[:, k, :], wh_sb[:, k, :],
                                start=(k == 0), stop=False,
                            )
                            nc.tensor.matmul(
                                ps[:], xh_sb[:, k + 1, :], wh_sb[:, k + 1, :],
                                start=False, stop=(k + 1 == KC - 1),
                            )
                            nc.tensor.matmul(
                                psc[:], x8_sb[:, :, k, :], w8_sb[:, :, k, :],
                                start=(k == 0), stop=False,
                                perf_mode=mybir.MatmulPerfMode.DoubleRow,
                            )
                            nc.tensor.matmul(
                                psc[:], x8_sb[:, :, k + 1, :], w8_sb[:, :, k + 1, :],
                                start=False, stop=(k + 1 == KC - 1),
                                perf_mode=mybir.MatmulPerfMode.DoubleRow,
                            )
                    else:
                        for k in range(KC):
                            nc.tensor.matmul(
                                ps[:],
                                xh_sb[:, k, :],
                                wh_sb[:, k, :],
                                start=(k == 0),
                                stop=(k == KC - 1),
                            )
                            nc.tensor.matmul(
                                psc[:],
                                x8_sb[:, :, k, :],
                                w8_sb[:, :, k, :],
                                start=(k == 0),
                                stop=(k == KC - 1),
                                perf_mode=mybir.MatmulPerfMode.DoubleRow,
                            )
                    ps2 = None
                    if PROBE == "dr2x":
                        ps2 = ps_pool.tile([P, NEXP], f32, tag="ps2", bufs=2)
                        for k in range(KC):
                            nc.tensor.matmul(
                                ps2[:],
                                x8_sb[:, :, k, :],
                                w8_sb[:, :, k, :],
                                start=(k == 0),
                                stop=(k == KC - 1),
                                perf_mode=mybir.MatmulPerfMode.DoubleRow,
                            )
                    elif PROBE == "main2x":
                        ps2 = ps_pool.tile([P, NEXP], f32, tag="ps2", bufs=2)
                        for k in range(KC):
                            nc.tensor.matmul(
                                ps2[:],
                                xh_sb[:, k, :],
                                wh_sb[:, k, :],
                                start=(k == 0),
                                stop=(k == KC - 1),
                            )
                else:
                    (xt_sb,) = loaded.pop(0)
                    for k in range(KC):
                        nc.tensor.matmul(
                            ps[:],
                            xt_sb[:, k, :],
                            wt_sb[:, k, :],
                            start=(k == 0),
                            stop=(k == KC - 1),
                        )

                if PROBE == "noepi" and hyb:
                    # epilogue-interference probe: bare PSUM evacuation only
                    corr = ep_pool.tile([P, NEXP], f32, tag="corr")
                    nc.vector.tensor_scalar(
                        corr[:], psc[:], 1.0 / S_WL, None, op0=mybir.AluOpType.mult
                    )
                    comb = ep_pool.tile([P, NEXP], f32, tag="comb")
                    nc.vector.tensor_add(comb[:], ps[:], corr[:])
                    nc.sync.dma_start(ow_d[b], comb[:, :TOPK])
                    nc.sync.dma_start(oi_d[b], comb[:, TOPK : 2 * TOPK].bitcast(u32))
                    continue
                if hyb:
                    # correction /= S_WL, then add main.  Only one PSUM
                    # operand allowed per DVE op: stage psc/S_WL in SBUF.
                    corr = ep_pool.tile([P, NEXP], f32, tag="corr")
                    nc.vector.tensor_scalar(
                        corr[:],
                        psc[:],
                        1.0 / S_WL,
                        None,
                        op0=mybir.AluOpType.mult,
                    )
                    comb = ep_pool.tile([P, NEXP], f32, tag="comb")
                    nc.vector.tensor_add(comb[:], ps[:], corr[:])
                    if ps2 is not None:
                        # fold the probe pass in at negligible weight so it
                        # is consumed (no DCE) without changing the values
                        corr2 = ep_pool.tile([P, NEXP], f32, tag="corr2")
                        nc.vector.tensor_scalar(
                            corr2[:], ps2[:], 1e-30, None, op0=mybir.AluOpType.mult
                        )
                        comb2 = ep_pool.tile([P, NEXP], f32, tag="comb2")
                        nc.vector.tensor_add(comb2[:], comb[:], corr2[:])
                        comb = comb2
                    sig_in = comb
                else:
                    sig_in = ps
                sig = ep_pool.tile([P, NEXP], f32, tag="sig")
                nc.scalar.activation(
                    sig[:],
                    sig_in[:],
                    mybir.ActivationFunctionType.Sigmoid,
                    scale=sig_scale,
                )

                biased = ep_pool.tile([P, NEXP], f32, tag="biased")
                nc.vector.tensor_add(biased[:], sig[:], bb_sb[:])

                max8 = ep_pool.tile([P, TOPK], f32, tag="max8")
                nc.vector.max(out=max8[:], in_=biased[:])
                idx = ep_pool.tile([P, TOPK], u32, tag="idx")
                nc.vector.max_index(out=idx[:], in_max=max8[:], in_values=biased[:])

                # Gather original sigmoid scores at the selected experts:
                # sel[:, j] = sum_e (biased[:, e] == max8[:, j]) * sig[:, e]
                sel = ep_pool.tile([P, TOPK], f32, tag="sel")
                scratch = ep_pool.tile([P, NEXP], f32, tag="scratch")
                for j in range(TOPK):
                    nc.vector.scalar_tensor_tensor(
                        out=scratch[:],
                        in0=biased[:],
                        scalar=max8[:, j : j + 1],
                        in1=sig[:],
                        op0=mybir.AluOpType.is_equal,
                        op1=mybir.AluOpType.mult,
                        accum_out=sel[:, j : j + 1],
                    )

                ssum = ep_pool.tile([P, 1], f32, tag="ssum")
                nc.vector.tensor_reduce(
                    ssum[:], sel[:], axis=mybir.AxisListType.X, op=mybir.AluOpType.add
                )
                rec = ep_pool.tile([P, 1], f32, tag="rec")
                nc.vector.reciprocal(rec[:], ssum[:])

                wout = ep_pool.tile([P, TOPK], f32, tag="wout")
                nc.vector.tensor_scalar(
                    wout[:],
                    sel[:],
                    rec[:],
                    ROUTE_SCALE,
                    op0=mybir.AluOpType.mult,
                    op1=mybir.AluOpType.mult,
                )

                # keep outputs on the sync queue: running anything on the
                # gpsimd queue costs ~4.4ms of fixed per-execution startup
                nc.sync.dma_start(ow_d[b], wout[:])
                nc.sync.dma_start(oi_d[b], idx[:])

    nc.compile()
    return nc


def _tile_x(x_shard):
    # [2048, D] -> [16, 128(tok), 56(d_out), 128(d_in)] -> [16, 128(d_in), 56, 128(tok)]
    return x_shard.reshape(NTILES, P, KC, P).transpose(0, 3, 2, 1)


def _prep_core_inputs(x_shard, wt, bb):
    if MM_DTYPE in ("k3", "hyb4"):
        import ml_dtypes

        f8 = ml_dtypes.float8_e4m3
        xs = (x_shard * X_SCALE).astype(np.float32)
        xh = xs.astype(np.float16)
        xl = xs - xh.astype(np.float32)
        out = {
            "xh": np.ascontiguousarray(_tile_x(xh)),
            "xl8": np.ascontiguousarray(_tile_x((xl * S_XL).astype(f8))),
            "wh": wt[0],
            "w8": wt[1],
            "bb": bb,
        }
        if MM_DTYPE == "hyb4":
            out["xh8"] = np.ascontiguousarray(_tile_x(xh.astype(f8)))
        return out
    return {"xt": np.ascontiguousarray(_tile_x(x_shard)), "wt": wt, "bb": bb}


def _prep_all(x, w, bias):
    # weight [256, 7168] -> [128(d_in), 56(d_out), 256(exp)]
    def _tile_w(warr):
        return np.ascontiguousarray(warr.reshape(NEXP, KC, P).transpose(2, 1, 0))

    if MM_DTYPE in ("k3", "hyb4"):
        import ml_dtypes

        f8 = ml_dtypes.float8_e4m3
        ws = (w * W_SCALE).astype(np.float32)
        wh = ws.astype(np.float16)
        wl = ws - wh.astype(np.float32)
        wl8 = _tile_w((wl * S_WL).astype(f8))             # pairs fp8(xh)
        wh8 = _tile_w((ws * S_WH).astype(f8))             # pairs xl8
        w8 = np.ascontiguousarray(np.stack([wl8, wh8], axis=1))
        wt = (_tile_w(wh), w8)
    else:
        wt = _tile_w(w)
    bb = np.ascontiguousarray(np.broadcast_to(bias, (P, NEXP)))

    with ThreadPoolExecutor(NCORES) as pool:
        return list(
            pool.map(
                lambda c: _prep_core_inputs(x[c * TPC : (c + 1) * TPC], wt, bb),
                range(NCORES),
            )
        )


def _collect(results):
    weights = np.concatenate(
        [r["out_w"].reshape(TPC, TOPK) for r in results], axis=0
    ).astype(np.float32)
    indices = np.concatenate(
        [r["out_i"].reshape(TPC, TOPK) for r in results], axis=0
    ).astype(np.int32)
    return weights, indices


def kernel(**inputs):
    from concourse.bass_utils import run_bass_kernel_spmd

    x = np.ascontiguousarray(np.asarray(inputs["x"], dtype=np.float32))
    w = np.ascontiguousarray(np.asarray(inputs["weight"], dtype=np.float32))
    bias = np.asarray(inputs["bias"], dtype=np.float32)

    in_maps = _prep_all(x, w, bias)
    nc = _build_program()
    res = run_bass_kernel_spmd(nc, in_maps, core_ids=list(range(NCORES)), trace=False)
    return _collect(res.results)



# revision 9
# speedup vs baseline: 1.4590x; 1.4590x over previous
"""MoE router gate kernel for Trainium2 (Bass/Tile), 8-core data-parallel,
two-phase (screen + selective rescore) implementation.

Computes, for x[16384, 7168], weight[256, 7168], bias[256]:
    scores  = sigmoid(x @ weight.T)
    biased  = scores + bias
    indices = top8(biased)                        (descending, int32)
    weights = scores[indices] / sum * 2.5         (float32)

Sharding: data-parallel over tokens (2048 tokens/core = 16 tiles of 128),
weight/bias replicated.

Two device programs per call:

  P1 (screen): fp16 main matmul only (xh = fp16(x*16), wh = fp16(w*64); the
  fp16 products accumulate exactly in fp32 PSUM, so score error is the
  representation error ~2^-11.5 in pre-sigmoid units).  Epilogue computes the
  full top-8 weights/indices for every token PLUS an ambiguity measure per
  token: the minimum consecutive gap among the top-9 biased scores (internal
  top-8 order swaps corrupt the index output too, so all eight boundaries
  matter, not just 8-vs-9).  Per 128-token tile the 16 smallest-gap tokens
  are selected on-device (PE transpose of the gap column + two DVE max8
  rounds) and exported as a map, together with the raw fp32 PSUM scores.

  P2 (rescore): host gathers the selected 256 tokens/core worth of fp8 data
  (fp8(xh) and fp8 of the x residual, from the prep arrays; no device gather
  -- register-offset APs crash this runtime) and P2 computes the fp8
  DoubleRow correction fp8(xh)*wl8 + xl8*wh8 for just those tokens against
  all 256 experts ([exp, slot] orientation, weights stationary), adds it to
  the gathered raw scores, and redoes sigmoid/top-8.  Host overwrites the
  rescored rows.  Rescored rows have exactly the old full-k3 accuracy
  (~2^-15), and the numpy simulation of this pipeline reproduces the full-k3
  error (10/16384 mismatched rows, rel err 4.6e-3) at cap=16 per tile.

  PE cost: P1 = 16 tiles * 56 chunks * 256 moving cols = 229376 cyc
  (~115us at ~2GHz); P2 = 2 halves * 56 chunks * 256 DR cols = 28672 cyc
  (~14us) + 8 small transposes.  The old single-program kernel streamed the
  correction for every token and was PE-bound at ~458752 cyc (~229us).
  DMA drops from 3B to 2B per x element (xl8 never ships in full).
"""

import os
from concurrent.futures import ThreadPoolExecutor

import numpy as np

TOKENS = 16384
DIM = 7168
NEXP = 256
TOPK = 8
ROUTE_SCALE = 2.5
NCORES = 8
TPC = TOKENS // NCORES          # tokens per core: 2048
P = 128                         # partitions / tile height
NTILES = TPC // P               # 16 token tiles per core
KC = DIM // P                   # 56 contraction chunks
CAP = 16                        # rescored tokens per tile
NSLOT = NTILES * CAP            # rescored tokens per core: 256

X_SCALE = 16.0   # keep x_lo out of fp16-denormal range
W_SCALE = 64.0   # keep w_lo out of fp16-denormal range
S_XL = 512.0     # scale of fp8(x residual)
S_WH = 8.0       # scale of fp8(w)
S_WL = S_XL * S_WH * 1.0  # scale of fp8(w residual); must equal S_XL*S_WH
SIG_SCALE = 1.0 / (X_SCALE * W_SCALE)

XBUFS = int(os.environ.get("GATE_XBUFS", "3"))
LOOKAHEAD = int(os.environ.get("GATE_LOOKAHEAD", "2"))
PSBUFS = int(os.environ.get("GATE_PSBUFS", "4"))


def _build_p1(reps=1):
    """Screen pass: fp16 scores, per-token top-8 + ambiguity selection."""
    import concourse.bacc as bacc
    import concourse.mybir as mybir
    import concourse.tile as tile

    f32 = mybir.dt.float32
    f16 = mybir.dt.float16
    u32 = mybir.dt.uint32

    nc = bacc.Bacc(
        "TRN2",
        target_bir_lowering=False,
        debug=False,
        enable_asserts=False,
        num_devices=NCORES,
    )

    xh_d = nc.dram_tensor("xh", [NTILES, P, KC, P], f16, kind="ExternalInput").ap()
    wh_d = nc.dram_tensor("wh", [P, KC, NEXP], f16, kind="ExternalInput").ap()
    bb_d = nc.dram_tensor("bb", [P, NEXP], f32, kind="ExternalInput").ap()
    id_d = nc.dram_tensor("ident", [P, P], f32, kind="ExternalInput").ap()
    ow_d = nc.dram_tensor("out_w", [NTILES, P, TOPK], f32, kind="ExternalOutput").ap()
    oi_d = nc.dram_tensor("out_i", [NTILES, P, TOPK], u32, kind="ExternalOutput").ap()
    sr_d = nc.dram_tensor("sraw", [NTILES, P, NEXP], f32, kind="ExternalOutput").ap()
    om_d = nc.dram_tensor("out_map", [NTILES, CAP], u32, kind="ExternalOutput").ap()

    with tile.TileContext(nc) as tc:
        with (
            tc.tile_pool(name="const", bufs=1) as const_pool,
            tc.tile_pool(name="xin", bufs=XBUFS) as x_pool,
            tc.tile_pool(name="psum", bufs=PSBUFS, space="PSUM") as ps_pool,
            tc.tile_pool(name="epi", bufs=3) as ep_pool,
        ):
            wh_sb = const_pool.tile([P, KC, NEXP], f16)
            nc.sync.dma_start(wh_sb[:], wh_d)
            bb_sb = const_pool.tile([P, NEXP], f32)
            nc.sync.dma_start(bb_sb[:], bb_d)
            id_sb = const_pool.tile([P, P], f32)
            nc.sync.dma_start(id_sb[:], id_d)
            # one negated-min-gap column per tile (last rep's values win)
            gapcol = const_pool.tile([P, NTILES], f32)

            seq = [b for _ in range(reps) for b in range(NTILES)]
            loaded = []

            def issue_load(b):
                xh_sb = x_pool.tile([P, KC, P], f16, tag="xh")
                nc.sync.dma_start(xh_sb[:], xh_d[b])
                loaded.append(xh_sb)

            def emit_selection():
                # per-tile top-CAP ambiguous-token selection: transpose the
                # key columns so each tile's 128 keys lie on the free axis of
                # one partition, then two max8 rounds select the 16 largest
                # keys (= smallest gaps) per tile in parallel.
                keyT_ps = ps_pool.tile([NTILES, P], f32, tag="keyT", bufs=1)
                nc.tensor.transpose(keyT_ps[:], gapcol[:], id_sb[:])
                keyT = ep_pool.tile([NTILES, P], f32, tag="keyT_sb")
                nc.vector.tensor_copy(keyT[:], keyT_ps[:])
                map_sb = ep_pool.tile([NTILES, CAP], u32, tag="map")
                k8 = ep_pool.tile([NTILES, 8], f32, tag="k8")
                nc.vector.max(out=k8[:], in_=keyT[:])
                nc.vector.max_index(
                    out=map_sb[:, 0:8], in_max=k8[:], in_values=keyT[:]
                )
                keyT2 = ep_pool.tile([NTILES, P], f32, tag="keyT2")
                nc.vector.match_replace(keyT2[:], k8[:], keyT[:], -1e30)
                k8b = ep_pool.tile([NTILES, 8], f32, tag="k8b")
                nc.vector.max(out=k8b[:], in_=keyT2[:])
                nc.vector.max_index(
                    out=map_sb[:, 8:16], in_max=k8b[:], in_values=keyT2[:]
                )
                nc.sync.dma_start(om_d, map_sb[:])

            for j in range(min(LOOKAHEAD, len(seq))):
                issue_load(seq[j])
            for i, b in enumerate(seq):
                if i + LOOKAHEAD < len(seq):
                    issue_load(seq[i + LOOKAHEAD])
                xh_sb = loaded.pop(0)
                ps = ps_pool.tile([P, NEXP], f32, tag="ps")
                for k in range(KC):
                    nc.tensor.matmul(
                        ps[:],
                        xh_sb[:, k, :],
                        wh_sb[:, k, :],
                        start=(k == 0),
                        stop=(k == KC - 1),
                    )

                # export raw fp32 scores for the host->P2 path
                sraw = ep_pool.tile([P, NEXP], f32, tag="sraw")
                nc.vector.tensor_copy(sraw[:], ps[:])
                nc.sync.dma_start(sr_d[b], sraw[:])

                sig = ep_pool.tile([P, NEXP], f32, tag="sig")
                nc.scalar.activation(
                    sig[:],
                    ps[:],
                    mybir.ActivationFunctionType.Sigmoid,
                    scale=SIG_SCALE,
                )
                biased = ep_pool.tile([P, NEXP], f32, tag="biased")
                nc.vector.tensor_add(biased[:], sig[:], bb_sb[:])

                m9 = ep_pool.tile([P, 9], f32, tag="m9")
                nc.vector.max(out=m9[:, 0:8], in_=biased[:])
                idx = ep_pool.tile([P, TOPK], u32, tag="idx")
                nc.vector.max_index(
                    out=idx[:], in_max=m9[:, 0:8], in_values=biased[:]
                )

                # 9th biased value -> min consecutive gap among top-9
                scr = ep_pool.tile([P, NEXP], f32, tag="scr")
                nc.vector.match_replace(scr[:], m9[:, 0:8], biased[:], -1e30)
                nc.vector.tensor_reduce(
                    m9[:, 8:9], scr[:], axis=mybir.AxisListType.X,
                    op=mybir.AluOpType.max,
                )
                gaps = ep_pool.tile([P, TOPK], f32, tag="gaps")
                nc.vector.tensor_sub(gaps[:], m9[:, 0:8], m9[:, 1:9])
                # negate while reducing: key = -mingap = max(-gaps)
                ngaps = ep_pool.tile([P, TOPK], f32, tag="ngaps")
                nc.vector.tensor_scalar(
                    ngaps[:], gaps[:], -1.0, None, op0=mybir.AluOpType.mult
                )
                nc.vector.tensor_reduce(
                    gapcol[:, b : b + 1], ngaps[:], axis=mybir.AxisListType.X,
                    op=mybir.AluOpType.max,
                )

                # weights: gather sigmoid scores at the selected experts
                sel = ep_pool.tile([P, TOPK], f32, tag="sel")
                scratch = ep_pool.tile([P, NEXP], f32, tag="scratch")
                for j in range(TOPK):
                    nc.vector.scalar_tensor_tensor(
                        out=scratch[:],
                        in0=biased[:],
                        scalar=m9[:, j : j + 1],
                        in1=sig[:],
                        op0=mybir.AluOpType.is_equal,
                        op1=mybir.AluOpType.mult,
                        accum_out=sel[:, j : j + 1],
                    )
                ssum = ep_pool.tile([P, 1], f32, tag="ssum")
                nc.vector.tensor_reduce(
                    ssum[:], sel[:], axis=mybir.AxisListType.X,
                    op=mybir.AluOpType.add,
                )
                rec = ep_pool.tile([P, 1], f32, tag="rec")
                nc.vector.reciprocal(rec[:], ssum[:])
                wout = ep_pool.tile([P, TOPK], f32, tag="wout")
                nc.vector.tensor_scalar(
                    wout[:],
                    sel[:],
                    rec[:],
                    ROUTE_SCALE,
                    op0=mybir.AluOpType.mult,
                    op1=mybir.AluOpType.mult,
                )
                nc.sync.dma_start(ow_d[b], wout[:])
                nc.sync.dma_start(oi_d[b], idx[:])
                if (i + 1) % NTILES == 0:
                    emit_selection()

    nc.compile()
    return nc


def _build_p2(reps=1):
    """Rescore pass: fp8 DoubleRow corrections for NSLOT gathered tokens."""
    import concourse.bacc as bacc
    import concourse.mybir as mybir
    import concourse.tile as tile

    f32 = mybir.dt.float32
    f8 = mybir.dt.float8e4
    u32 = mybir.dt.uint32

    nc = bacc.Bacc(
        "TRN2",
        target_bir_lowering=False,
        debug=False,
        enable_asserts=False,
        num_devices=NCORES,
    )

    # w8[:, 0] = wl8 (pairs fp8(xh)), w8[:, 1] = wh8 (pairs xl8)
    w8_d = nc.dram_tensor("w8", [P, 2, KC, NEXP], f8, kind="ExternalInput").ap()
    xg_d = nc.dram_tensor("x8g", [P, 2, KC, NSLOT], f8, kind="ExternalInput").ap()
    sg_d = nc.dram_tensor("sgT", [P, 2, NSLOT], f32, kind="ExternalInput").ap()
    bc_d = nc.dram_tensor("bias_col", [P, 2], f32, kind="ExternalInput").ap()
    id_d = nc.dram_tensor("ident", [P, P], f32, kind="ExternalInput").ap()
    ow_d = nc.dram_tensor("ow2", [2, P, TOPK], f32, kind="ExternalOutput").ap()
    oi_d = nc.dram_tensor("oi2", [2, P, TOPK], u32, kind="ExternalOutput").ap()

    NST = NSLOT // P  # slot tiles (2)

    with tile.TileContext(nc) as tc:
        with (
            tc.tile_pool(name="const", bufs=1) as const_pool,
            tc.tile_pool(name="psum", bufs=2, space="PSUM") as ps_pool,
            tc.tile_pool(name="epi", bufs=2) as ep_pool,
        ):
            w8_sb = const_pool.tile([P, 2, KC, NEXP], f8)
            nc.sync.dma_start(w8_sb[:], w8_d)
            xg_sb = const_pool.tile([P, 2, KC, NSLOT], f8)
            nc.sync.dma_start(xg_sb[:], xg_d)
            sg_sb = const_pool.tile([P, 2, NSLOT], f32)
            nc.sync.dma_start(sg_sb[:], sg_d)
            bc_sb = const_pool.tile([P, 2], f32)
            nc.sync.dma_start(bc_sb[:], bc_d)
            id_sb = const_pool.tile([P, P], f32)
            nc.sync.dma_start(id_sb[:], id_d)

            for _ in range(reps):
                sigT = [
                    ep_pool.tile([P, NEXP], f32, tag=f"sigT{st}",
                                 name=f"sigT{st}")
                    for st in range(NST)
                ]
                biasedT = [
                    ep_pool.tile([P, NEXP], f32, tag=f"bT{st}",
                                 name=f"bT{st}")
                    for st in range(NST)
                ]
                for h in range(2):
                    psc = ps_pool.tile([P, NSLOT], f32, tag="psc")
                    for k in range(KC):
                        nc.tensor.matmul(
                            psc[:],
                            w8_sb[:, :, k, h * P : (h + 1) * P],
                            xg_sb[:, :, k, :],
                            start=(k == 0),
                            stop=(k == KC - 1),
                            perf_mode=mybir.MatmulPerfMode.DoubleRow,
                        )
                    # s2 = sraw + corr/S_WL ; sig2 = sigmoid(s2/(16*64))
                    corr = ep_pool.tile([P, NSLOT], f32, tag="corr")
                    nc.vector.tensor_scalar(
                        corr[:], psc[:], 1.0 / S_WL, None, op0=mybir.AluOpType.mult
                    )
                    s2 = ep_pool.tile([P, NSLOT], f32, tag="s2")
                    nc.vector.tensor_add(s2[:], corr[:], sg_sb[:, h, :])
                    sig2 = ep_pool.tile([P, NSLOT], f32, tag="sig2")
                    nc.scalar.activation(
                        sig2[:],
                        s2[:],
                        mybir.ActivationFunctionType.Sigmoid,
                        scale=SIG_SCALE,
                    )
                    b2 = ep_pool.tile([P, NSLOT], f32, tag="b2")
                    nc.vector.tensor_scalar(
                        b2[:], sig2[:], bc_sb[:, h : h + 1], None,
                        op0=mybir.AluOpType.add,
                    )
                    # transpose [exp, slot] -> [slot, exp] per slot-tile;
                    # expert half h lands in columns [h*128, (h+1)*128)
                    for st in range(NST):
                        tp = ps_pool.tile([P, P], f32, tag="tp")
                        nc.tensor.transpose(
                            tp[:], sig2[:, st * P : (st + 1) * P], id_sb[:]
                        )
                        nc.vector.tensor_copy(
                            sigT[st][:, h * P : (h + 1) * P], tp[:]
                        )
                        tp2 = ps_pool.tile([P, P], f32, tag="tp2")
                        nc.tensor.transpose(
                            tp2[:], b2[:, st * P : (st + 1) * P], id_sb[:]
                        )
                        nc.vector.tensor_copy(
                            biasedT[st][:, h * P : (h + 1) * P], tp2[:]
                        )

                for st in range(NST):
                    bT = biasedT[st][:]
                    gT = sigT[st][:]
                    max8 = ep_pool.tile([P, TOPK], f32, tag="max8")
                    nc.vector.max(out=max8[:], in_=bT)
                    idx = ep_pool.tile([P, TOPK], u32, tag="idx")
                    nc.vector.max_index(out=idx[:], in_max=max8[:], in_values=bT)
                    sel = ep_pool.tile([P, TOPK], f32, tag="sel")
                    scratch = ep_pool.tile([P, NEXP], f32, tag="scratch")
                    for j in range(TOPK):
                        nc.vector.scalar_tensor_tensor(
                            out=scratch[:],
                            in0=bT,
                            scalar=max8[:, j : j + 1],
                            in1=gT,
                            op0=mybir.AluOpType.is_equal,
                            op1=mybir.AluOpType.mult,
                            accum_out=sel[:, j : j + 1],
                        )
                    ssum = ep_pool.tile([P, 1], f32, tag="ssum")
                    nc.vector.tensor_reduce(
                        ssum[:], sel[:], axis=mybir.AxisListType.X,
                        op=mybir.AluOpType.add,
                    )
                    rec = ep_pool.tile([P, 1], f32, tag="rec")
                    nc.vector.reciprocal(rec[:], ssum[:])
                    wout = ep_pool.tile([P, TOPK], f32, tag="wout")
                    nc.vector.tensor_scalar(
                        wout[:],
                        sel[:],
                        rec[:],
                        ROUTE_SCALE,
                        op0=mybir.AluOpType.mult,
                        op1=mybir.AluOpType.mult,
                    )
                    nc.sync.dma_start(ow_d[st], wout[:])
                    nc.sync.dma_start(oi_d[st], idx[:])

    nc.compile()
    return nc


def _tile_x(x_shard):
    # [2048, D] -> [16, 128(tok), 56(d_out), 128(d_in)] -> [16, 128(d_in), 56, 128(tok)]
    return x_shard.reshape(NTILES, P, KC, P).transpose(0, 3, 2, 1)


_IDENT = np.eye(P, dtype=np.float32)


def _prep_core(x_shard, wh_t, bb):
    """P1 inputs + host-side tiled arrays kept for the P2 gather."""
    import ml_dtypes

    f8 = ml_dtypes.float8_e4m3
    xs = (x_shard * X_SCALE).astype(np.float32)
    xh = xs.astype(np.float16)
    xl = xs - xh.astype(np.float32)
    xh_t = np.ascontiguousarray(_tile_x(xh))
    xl8_t = np.ascontiguousarray(_tile_x((xl * S_XL).astype(f8)))
    p1_in = {"xh": xh_t, "wh": wh_t, "bb": bb, "ident": _IDENT}
    return p1_in, xh_t, xl8_t


def _prep_all(x, w, bias):
    import ml_dtypes

    f8 = ml_dtypes.float8_e4m3

    def _tile_w(warr):
        # [256, 7168] -> [128(d_in), 56(d_out), 256(exp)]
        return np.ascontiguousarray(warr.reshape(NEXP, KC, P).transpose(2, 1, 0))

    ws = (w * W_SCALE).astype(np.float32)
    wh = ws.astype(np.float16)
    wl = ws - wh.astype(np.float32)
    wh_t = _tile_w(wh)
    wl8 = _tile_w((wl * S_WL).astype(f8))             # pairs fp8(xh)
    wh8 = _tile_w((ws * S_WH).astype(f8))             # pairs xl8
    w8 = np.ascontiguousarray(np.stack([wl8, wh8], axis=1))
    bb = np.ascontiguousarray(np.broadcast_to(bias, (P, NEXP)).astype(np.float32))
    # bias_col[p, h] = bias[h*128 + p]
    bias_col = np.ascontiguousarray(bias.reshape(2, P).T.astype(np.float32))

    with ThreadPoolExecutor(NCORES) as pool:
        cores = list(
            pool.map(
                lambda c: _prep_core(x[c * TPC : (c + 1) * TPC], wh_t, bb),
                range(NCORES),
            )
        )
    return cores, w8, bias_col


def _gather_p2_inputs(core_prep, p1_out, w8, bias_col):
    """Host gather of the selected tokens' fp8 data + raw scores."""
    import ml_dtypes

    f8 = ml_dtypes.float8_e4m3
    _, xh_t, xl8_t = core_prep
    m = np.asarray(p1_out["out_map"], np.int64)        # [16, 16] token-in-tile
    tiles = np.repeat(np.arange(NTILES), CAP)          # [256]
    toks = m.reshape(-1)                               # [256]
    # [256, 128, 56] -> [128, 56, 256]
    xh_g = xh_t[tiles, :, :, toks].transpose(1, 2, 0)
    xl8_g = xl8_t[tiles, :, :, toks].transpose(1, 2, 0)
    x8g = np.empty((P, 2, KC, NSLOT), f8)
    x8g[:, 0] = xh_g.astype(f8)
    x8g[:, 1] = xl8_g
    sraw = np.asarray(p1_out["sraw"])                  # [16, 128, 256]
    sg = sraw[tiles, toks]                             # [256 slots, 256 exp]
    sgT = np.ascontiguousarray(
        sg.T.reshape(2, P, NSLOT).transpose(1, 0, 2)
    )  # [128, 2, 256]
    return {
        "w8": w8,
        "x8g": np.ascontiguousarray(x8g),
        "sgT": sgT,
        "bias_col": bias_col,
        "ident": _IDENT,
    }, tiles, toks


def _merge(p1_results, p2_results, maps):
    weights = np.concatenate(
        [np.asarray(r["out_w"]).reshape(TPC, TOPK) for r in p1_results], axis=0
    ).astype(np.float32)
    indices = np.concatenate(
        [np.asarray(r["out_i"]).reshape(TPC, TOPK) for r in p1_results], axis=0
    ).astype(np.int32)
    for c, (r2, (tiles, toks)) in enumerate(zip(p2_results, maps)):
        rows = c * TPC + tiles * P + toks
        weights[rows] = np.asarray(r2["ow2"]).reshape(NSLOT, TOPK)
        indices[rows] = np.asarray(r2["oi2"]).reshape(NSLOT, TOPK).astype(np.int32)
    return weights, indices


def kernel(**inputs):
    from concourse.bass_utils import run_bass_kernel_spmd

    x = np.ascontiguousarray(np.asarray(inputs["x"], dtype=np.float32))
    w = np.ascontiguousarray(np.asarray(inputs["weight"], dtype=np.float32))
    bias = np.asarray(inputs["bias"], dtype=np.float32)

    cores, w8, bias_col = _prep_all(x, w, bias)

    nc1 = _build_p1()
    r1 = run_bass_kernel_spmd(
        nc1, [c[0] for c in cores], core_ids=list(range(NCORES)), trace=False
    ).results

    p2_maps = []
    p2_ins = []
    for c in range(NCORES):
        p2_in, tiles, toks = _gather_p2_inputs(cores[c], r1[c], w8, bias_col)
        p2_ins.append(p2_in)
        p2_maps.append((tiles, toks))

    nc2 = _build_p2()
    r2 = run_bass_kernel_spmd(
        nc2, p2_ins, core_ids=list(range(NCORES)), trace=False
    ).results

    return _merge(r1, r2, p2_maps)
